# revision 20
# baseline (speedup 1.0000x reference)
"""Trainium2 Bass kernel: per-tensor asymmetric int8 activation quantization
followed by a linear layer (y = quantize(x) @ W.T + bias).

Sharding (8 cores): 4-way over tokens x 2-way over out_features.
Each core receives:
  xT   [D_IN, TOK_C]   fp32  (x transposed, token-sharded)
  wT   [D_IN, DOUT_C]  fp16  (W transposed, out_feature-sharded)
  bias [DOUT_C]        fp16
and produces y [TOK_C, DOUT_C] fp32.

Device program per core:
  phase 0: streaming min/max over the local x shard (DVE reduce + GPSIMD
           partition all-reduce), then an 8-core AllReduce(max) of
           [xmax, -xmin] to get the global per-tensor range.
  scalars: inv_scale = 255/(xmax-xmin); zp = clip(-128 - rne(xmin/scale));
           rne() implemented with the fp32 magic constant 1.5*2^23.
  main:    for each 128-token block: quantize (ACT fused scale+magic, DVE
           zero-point + clip, cast fp16 -- q is integer in [-128,127] so
           fp16 is exact), then fp16 matmuls accumulate fp32 into PSUM with
           the weight tensor resident in SBUF; bias is folded in as a K=1
           matmul against a ones vector; result DMAd out in natural
           [token, dout] layout.
"""

import sys

import numpy as np

try:  # the grading environment may or may not have concourse on sys.path
    import concourse  # noqa: F401
except ImportError:  # pragma: no cover
    sys.path.insert(0, "/opt/trn_rl_repo")

P = 128
MAGIC = 12582912.0  # 1.5 * 2**23: fp32 add/sub rounds to nearest-even integer
QMIN, QMAX = -128.0, 127.0

# Full-problem shape (hardcoded per contract; kernel() checks them)
B, S, D_IN, D_OUT = 4, 2048, 4096, 4096
R_SHARDS, G_SHARDS = 4, 2  # token shards x out_feature shards
N_CORES = 8


def build_program(d_in, tok, dout, n_cores=N_CORES, w_passes=1, bias_mode="matmul"):
    """Emit the per-core SPMD program. Returns a compiled Bacc object.

    w_passes=2 adds a second accumulation pass against a residual weight
    input ("wLo") for near-fp32 weight precision at 2x PE cost.
    bias_mode: "matmul" folds bias in as a K=1 matmul; "evict" adds it
    during PSUM eviction on the vector engine (no K=1 weight loads).
    """
    from contextlib import ExitStack

    import concourse.bacc as bacc
    import concourse.tile as tile
    from concourse import bass_isa, mybir

    f32, f16 = mybir.dt.float32, mybir.dt.float16
    AF = mybir.ActivationFunctionType
    ALU = mybir.AluOpType
    AX = mybir.AxisListType

    assert d_in % P == 0 and tok % P == 0
    assert tok <= dout  # phase-0 reuses the [P, dout] output-pool slots
    KB, MB = d_in // P, tok // P
    KB0 = KB // 2  # phase-0 min/max half (the other half is on the g-sibling core)
    NMM = min(512, dout)
    assert dout % NMM == 0
    NB = dout // NMM

    nc = bacc.Bacc(
        "TRN2",
        target_bir_lowering=False,
        debug=False,
        num_devices=n_cores,
        enable_asserts=False,
    )

    bias_dt = f16 if bias_mode == "matmul" else f32
    xT = nc.dram_tensor("xT", [d_in, tok], f32, kind="ExternalInput").ap()
    wT = nc.dram_tensor("wT", [d_in, dout], f16, kind="ExternalInput").ap()
    bias = nc.dram_tensor("bias", [dout], bias_dt, kind="ExternalInput").ap()
    w_ins = [wT]
    if w_passes == 2:
        w_ins.append(nc.dram_tensor("wLo", [d_in, dout], f16, kind="ExternalInput").ap())
    y = nc.dram_tensor("y", [tok, dout], f32, kind="ExternalOutput").ap()
    cc_in = nc.dram_tensor("cc_in", [2], f32).ap()
    cc_out = nc.dram_tensor("cc_out", [2], f32, addr_space="Shared").ap()

    x_view = xT.rearrange("(kb p) t -> p kb t", p=P)  # [P, KB, tok]
    w_views = [w.rearrange("(kb p) o -> p kb o", p=P) for w in w_ins]

    with tile.TileContext(nc) as tc, ExitStack() as ctx:
        wpool = ctx.enter_context(tc.tile_pool(name="w", bufs=1))
        xpool = ctx.enter_context(tc.tile_pool(name="x", bufs=2))
        qpool = ctx.enter_context(tc.tile_pool(name="q", bufs=2))
        obufs = 3 if bias_mode == "matmul" else 2
        opool = ctx.enter_context(tc.tile_pool(name="o", bufs=obufs))
        spool = ctx.enter_context(tc.tile_pool(name="s", bufs=1))
        ppool = ctx.enter_context(tc.tile_pool(name="ps", bufs=2, space="PSUM"))

        # ---- phase 0: min/max over the first half of this core's x shard
        # (the g-sibling core covers the other half; the host rolls the d_in
        # axis for g=1 cores so "first half" differs between siblings).
        # Full-row tiles: 8KB contiguous per partition -> full DMA rate.
        smax = spool.tile([P, KB0], f32)
        smin = spool.tile([P, KB0], f32)
        ph0_dmas = []
        for kb in range(KB0):
            x_p = opool.tile([P, dout], f32, tag="o_m")
            x_row = x_p[:, 0:tok]
            ph0_dmas.append(nc.sync.dma_start(x_row, x_view[:, kb, :]))
            nc.vector.tensor_reduce(smax[:, kb : kb + 1], x_row, axis=AX.X, op=ALU.max)
            nc.vector.tensor_reduce(smin[:, kb : kb + 1], x_row, axis=AX.X, op=ALU.min)

        # Resident weights on the Scalar engine's HWDGE queue, staggered
        # behind phase-0's x traffic so they don't compete for HBM fabric.
        w_sbs = []
        n_chunks = 4
        step = max(1, KB // n_chunks)
        chunk_no = 0
        for wi, wv in enumerate(w_views):
            w_sb = wpool.tile([P, KB, dout], f16, tag=f"wsb{wi}")
            for k0 in range(0, KB, step):
                k1 = min(KB, k0 + step)
                gate_idx = min((chunk_no + 1) * max(1, KB0 // n_chunks), KB0) - 1
                wdma = nc.scalar.dma_start(w_sb[:, k0:k1, :], wv[:, k0:k1, :])
                tile.add_dep_helper(
                    wdma.ins, ph0_dmas[gate_idx].ins, reason="stagger W behind phase-0 x"
                )
                chunk_no += 1
            w_sbs.append(w_sb)
        if bias_mode == "matmul":
            bias_row = wpool.tile([1, dout], f16)
            nc.scalar.dma_start(bias_row[:], bias[None, :])
            ones_t = wpool.tile([1, P], f16)
            nc.vector.memset(ones_t[:], 1.0)
        else:
            bias_bc = wpool.tile([P, dout], f32)
            nc.scalar.dma_start(bias_bc[0:1, :], bias[None, :])
            nc.gpsimd.partition_broadcast(bias_bc[:], bias_bc[0:1, :], channels=P)

        pk = spool.tile([P, 2], f32)
        nc.vector.tensor_reduce(pk[:, 0:1], smax[:], axis=AX.X, op=ALU.max)
        nc.vector.tensor_reduce(pk[:, 1:2], smin[:], axis=AX.X, op=ALU.min)
        nc.vector.tensor_scalar_mul(pk[:, 1:2], pk[:, 1:2], -1.0)
        pkr = spool.tile([P, 2], f32)
        nc.gpsimd.partition_all_reduce(
            pkr[:], pk[:], channels=P, reduce_op=bass_isa.ReduceOp.max
        )

        # ---- 8-core AllReduce(max) of [xmax, -xmin] ----
        sc = spool.tile([1, 2], f32)
        sem_in = nc.alloc_semaphore("ar_in")
        sem_cc = nc.alloc_semaphore("ar_cc")
        sem_out = nc.alloc_semaphore("ar_out")
        with tc.tile_critical():
            nc.gpsimd.dma_start(cc_in[None, :], pkr[0:1, :]).then_inc(sem_in, 16)
            nc.gpsimd.wait_ge(sem_in, 16)
            nc.gpsimd.collective_compute(
                "AllReduce",
                ALU.max,
                replica_groups=[list(range(n_cores))],
                ins=[cc_in],
                outs=[cc_out],
            ).then_inc(sem_cc, 1)
            nc.gpsimd.wait_ge(sem_cc, 1)
            nc.gpsimd.dma_start(sc[:], cc_out[None, :]).then_inc(sem_out, 16)
            nc.gpsimd.wait_ge(sem_out, 16)

        # ---- scalar math: inv_scale, zp ----
        scr = spool.tile([1, 6], f32)
        rng, inv, isc, nt, zp, mzp = (scr[0:1, i : i + 1] for i in range(6))
        nc.vector.tensor_add(rng, sc[0:1, 0:1], sc[0:1, 1:2])  # xmax - xmin
        nc.vector.reciprocal(inv, rng)
        nc.vector.tensor_scalar_mul(isc, inv, 255.0)  # 255/(xmax-xmin) ~ 1/scale
        nc.vector.tensor_mul(nt, sc[0:1, 1:2], isc)  # (-xmin)/scale
        # rne(nt); then zp = clip(-128 + rne(nt), -128, 127)
        nc.vector.tensor_scalar(zp, nt, MAGIC, -MAGIC, op0=ALU.add, op1=ALU.add)
        nc.vector.tensor_scalar(zp, zp, -128.0, -128.0, op0=ALU.add, op1=ALU.max)
        nc.vector.tensor_scalar_min(zp, zp, 127.0)
        nc.vector.tensor_scalar(mzp, zp, -1.0, MAGIC, op0=ALU.mult, op1=ALU.add)
        bc0 = spool.tile([1, 2], f32)
        nc.vector.tensor_copy(bc0[0:1, 0:1], isc)
        nc.vector.tensor_copy(bc0[0:1, 1:2], mzp)
        bc = spool.tile([P, 2], f32)
        nc.gpsimd.partition_broadcast(bc[:], bc0[:], channels=P)

        # ---- main loop: quantize + matmul per 128-token block ----
        for mb in range(MB):
            x_m = xpool.tile([P, KB * P], f32, tag="xm")
            x_m3 = x_m.rearrange("p (a b) -> p a b", b=P)  # [P, KB, P]
            nc.sync.dma_start(x_m3, x_view[:, :, mb * P : (mb + 1) * P])
            # v = x * inv_scale + MAGIC  (ACT); upper bits now hold rne(x/scale)
            nc.scalar.activation(x_m[:], x_m[:], AF.Copy, bias=MAGIC, scale=bc[:, 0:1])
            # v - (MAGIC - zp) = rne(x/scale) + zp ; clamp low
            nc.vector.tensor_scalar(
                x_m[:], x_m[:], bc[:, 1:2], QMIN, op0=ALU.subtract, op1=ALU.max
            )
            q_m = qpool.tile([P, KB, P], f16)
            nc.vector.tensor_scalar(q_m[:], x_m3, QMAX, None, op0=ALU.min)

            psum = ppool.tile([P, dout], f32)
            if bias_mode == "matmul":
                for n in range(NB):
                    nc.tensor.matmul(
                        psum[:, n * NMM : (n + 1) * NMM],
                        ones_t[:],
                        bias_row[:, n * NMM : (n + 1) * NMM],
                        start=True,
                        stop=False,
                    )
            last_wi = len(w_sbs) - 1
            for wi, w_sb in enumerate(w_sbs):
                for kb in range(KB):
                    lhsT = q_m[:, kb, :]
                    for n in range(NB):
                        nc.tensor.matmul(
                            psum[:, n * NMM : (n + 1) * NMM],
                            lhsT,
                            w_sb[:, kb, n * NMM : (n + 1) * NMM],
                            start=(bias_mode != "matmul" and wi == 0 and kb == 0),
                            stop=(kb == KB - 1 and wi == last_wi),
                        )
            o_m = opool.tile([P, dout], f32, tag="o_m")
            if bias_mode == "matmul":
                nc.scalar.copy(o_m[:], psum[:])
            else:
                nc.vector.scalar_tensor_tensor(
                    o_m[:], psum[:], 1.0, bias_bc[:], op0=ALU.mult, op1=ALU.add
                )
            nc.gpsimd.dma_start(y[mb * P : (mb + 1) * P, :], o_m[:])

    nc.compile()
    return nc


def make_in_maps(
    x, weight, bias, r_shards=R_SHARDS, g_shards=G_SHARDS, w_passes=1, bias_mode="matmul"
):
    """Host-side shard/layout prep. Returns (in_maps, tok_c, dout_c)."""
    x = np.asarray(x, dtype=np.float32)
    weight = np.asarray(weight, dtype=np.float32)
    bias = np.asarray(bias, dtype=np.float32)
    tok_tot = int(np.prod(x.shape[:-1]))
    d_in = x.shape[-1]
    d_out = weight.shape[0]
    tok_c = tok_tot // r_shards
    dout_c = d_out // g_shards

    xt = np.ascontiguousarray(x.reshape(tok_tot, d_in).T)  # [d_in, tok_tot]
    b16 = bias.astype(np.float16) if bias_mode == "matmul" else bias.astype(np.float32)
    # g=1 cores get the d_in axis rolled by half so the SPMD program's
    # phase-0 min/max pass (which always scans the first d_in/2 rows) covers
    # the other half of x on the sibling core. Contraction order is
    # irrelevant to the matmul as long as xT and wT are rolled identically.
    half = d_in // 2

    def _roll(a, g):
        return a if g % 2 == 0 else np.concatenate([a[half:], a[:half]], axis=0)

    w_hi, w_lo = [], []
    for g in range(g_shards):
        wg = weight[g * dout_c : (g + 1) * dout_c, :].T  # [d_in, dout_c] fp32
        wg = _roll(wg, g)
        hi = wg.astype(np.float16)
        w_hi.append(np.ascontiguousarray(hi))
        if w_passes == 2:
            w_lo.append(np.ascontiguousarray((wg - hi.astype(np.float32)).astype(np.float16)))

    in_maps = []
    for c in range(r_shards * g_shards):
        r, g = divmod(c, g_shards)
        m = {
            "xT": np.ascontiguousarray(_roll(xt[:, r * tok_c : (r + 1) * tok_c], g)),
            "wT": w_hi[g],
            "bias": np.ascontiguousarray(b16[g * dout_c : (g + 1) * dout_c]),
        }
        if w_passes == 2:
            m["wLo"] = w_lo[g]
        in_maps.append(m)
    return in_maps, tok_c, dout_c


def assemble_output(results, out_shape, tok_c, dout_c, g_shards=G_SHARDS):
    d_out = out_shape[-1]
    tok_tot = int(np.prod(out_shape[:-1]))
    Y = np.empty((tok_tot, d_out), np.float32)
    for c, res in enumerate(results):
        r, g = divmod(c, g_shards)
        Y[r * tok_c : (r + 1) * tok_c, g * dout_c : (g + 1) * dout_c] = res["y"]
    return Y.reshape(out_shape)


_PROGRAM_CACHE = {}


def _get_program(d_in, tok_c, dout_c, w_passes, bias_mode):
    key = (d_in, tok_c, dout_c, w_passes, bias_mode)
    if key not in _PROGRAM_CACHE:
        _PROGRAM_CACHE[key] = build_program(
            d_in, tok_c, dout_c, N_CORES, w_passes, bias_mode
        )
    return _PROGRAM_CACHE[key]


def kernel(x, weight, bias, w_passes=1, bias_mode="matmul", trace=False):
    """Full-input entry point: shards across 8 NeuronCores, runs, gathers."""
    from concourse.bass_utils import run_bass_kernel_spmd

    assert x.shape == (B, S, D_IN) and weight.shape == (D_OUT, D_IN)
    in_maps, tok_c, dout_c = make_in_maps(
        x, weight, bias, w_passes=w_passes, bias_mode=bias_mode
    )
    nc = _get_program(D_IN, tok_c, dout_c, w_passes, bias_mode)
    out = run_bass_kernel_spmd(nc, in_maps, list(range(N_CORES)), trace=trace)
    res = assemble_output(out.results, (B, S, D_OUT), tok_c, dout_c)
    if trace:
        return res, out
    return res


# revision 22
# speedup vs baseline: 1.0265x; 1.0265x over previous
"""Trainium2 Bass kernel: per-tensor asymmetric int8 activation quantization
followed by a linear layer (y = quantize(x) @ W.T + bias).

Sharding (8 cores): 4-way over tokens x 2-way over out_features.
Each core receives:
  xT   [D_IN, TOK_C]   fp32  (x transposed, token-sharded)
  wT   [D_IN, DOUT_C]  fp16  (W transposed, out_feature-sharded)
  bias [DOUT_C]        fp16
and produces y [TOK_C, DOUT_C] fp32.

Device program per core:
  phase 0: streaming min/max over the local x shard (DVE reduce + GPSIMD
           partition all-reduce), then an 8-core AllReduce(max) of
           [xmax, -xmin] to get the global per-tensor range.
  scalars: inv_scale = 255/(xmax-xmin); zp = clip(-128 - rne(xmin/scale));
           rne() implemented with the fp32 magic constant 1.5*2^23.
  main:    for each 128-token block: quantize (ACT fused scale+magic, DVE
           zero-point + clip, cast fp16 -- q is integer in [-128,127] so
           fp16 is exact), then fp16 matmuls accumulate fp32 into PSUM with
           the weight tensor resident in SBUF; bias is folded in as a K=1
           matmul against a ones vector; result DMAd out in natural
           [token, dout] layout.
"""

import sys

import numpy as np

try:  # the grading environment may or may not have concourse on sys.path
    import concourse  # noqa: F401
except ImportError:  # pragma: no cover
    sys.path.insert(0, "/opt/trn_rl_repo")

P = 128
MAGIC = 12582912.0  # 1.5 * 2**23: fp32 add/sub rounds to nearest-even integer
QMIN, QMAX = -128.0, 127.0

# Full-problem shape (hardcoded per contract; kernel() checks them)
B, S, D_IN, D_OUT = 4, 2048, 4096, 4096
R_SHARDS, G_SHARDS = 4, 2  # token shards x out_feature shards
N_CORES = 8


def build_program(d_in, tok, dout, n_cores=N_CORES, w_passes=1, bias_mode="matmul"):
    """Emit the per-core SPMD program. Returns a compiled Bacc object.

    w_passes=2 adds a second accumulation pass against a residual weight
    input ("wLo") for near-fp32 weight precision at 2x PE cost.
    bias_mode: "matmul" folds bias in as a K=1 matmul; "evict" adds it
    during PSUM eviction on the vector engine (no K=1 weight loads).
    """
    from contextlib import ExitStack

    import concourse.bacc as bacc
    import concourse.tile as tile
    from concourse import bass_isa, mybir

    f32, f16 = mybir.dt.float32, mybir.dt.float16
    AF = mybir.ActivationFunctionType
    ALU = mybir.AluOpType
    AX = mybir.AxisListType

    assert d_in % P == 0 and tok % P == 0
    assert tok <= dout  # phase-0 reuses the [P, dout] output-pool slots
    KB, MB = d_in // P, tok // P
    KB0 = KB // 2  # phase-0 min/max half (the other half is on the g-sibling core)
    NMM = min(512, dout)
    assert dout % NMM == 0
    NB = dout // NMM

    nc = bacc.Bacc(
        "TRN2",
        target_bir_lowering=False,
        debug=False,
        num_devices=n_cores,
        enable_asserts=False,
    )

    bias_dt = f16 if bias_mode == "matmul" else f32
    xT = nc.dram_tensor("xT", [d_in, tok], f32, kind="ExternalInput").ap()
    wT = nc.dram_tensor("wT", [d_in, dout], f16, kind="ExternalInput").ap()
    bias = nc.dram_tensor("bias", [dout], bias_dt, kind="ExternalInput").ap()
    w_ins = [wT]
    if w_passes == 2:
        w_ins.append(nc.dram_tensor("wLo", [d_in, dout], f16, kind="ExternalInput").ap())
    y = nc.dram_tensor("y", [tok, dout], f32, kind="ExternalOutput").ap()
    cc_in = nc.dram_tensor("cc_in", [2], f32).ap()
    cc_out = nc.dram_tensor("cc_out", [2], f32, addr_space="Shared").ap()

    x_view = xT.rearrange("(kb p) t -> p kb t", p=P)  # [P, KB, tok]
    w_views = [w.rearrange("(kb p) o -> p kb o", p=P) for w in w_ins]

    with tile.TileContext(nc) as tc, ExitStack() as ctx:
        wpool = ctx.enter_context(tc.tile_pool(name="w", bufs=1))
        xpool = ctx.enter_context(tc.tile_pool(name="x", bufs=2))
        qpool = ctx.enter_context(tc.tile_pool(name="q", bufs=2))
        obufs = 3 if bias_mode == "matmul" else 2
        opool = ctx.enter_context(tc.tile_pool(name="o", bufs=obufs))
        spool = ctx.enter_context(tc.tile_pool(name="s", bufs=1))
        ppool = ctx.enter_context(tc.tile_pool(name="ps", bufs=2, space="PSUM"))

        # ---- phase 0: min/max over the first half of this core's x shard
        # (the g-sibling core covers the other half; the host rolls the d_in
        # axis for g=1 cores so "first half" differs between siblings).
        # Full-row tiles: 8KB contiguous per partition -> full DMA rate.
        smax = spool.tile([P, KB0], f32)
        smin = spool.tile([P, KB0], f32)
        ph0_dmas = []
        for kb in range(KB0):
            x_p = opool.tile([P, dout], f32, tag="o_m")
            x_row = x_p[:, 0:tok]
            ph0_dmas.append(nc.sync.dma_start(x_row, x_view[:, kb, :]))
            nc.vector.tensor_reduce(smax[:, kb : kb + 1], x_row, axis=AX.X, op=ALU.max)
            nc.vector.tensor_reduce(smin[:, kb : kb + 1], x_row, axis=AX.X, op=ALU.min)

        # Resident weights on the Scalar engine's HWDGE queue, staggered
        # behind phase-0's x traffic so they don't compete for HBM fabric.
        w_sbs = []
        n_chunks = 4
        step = max(1, KB // n_chunks)
        chunk_no = 0
        for wi, wv in enumerate(w_views):
            w_sb = wpool.tile([P, KB, dout], f16, tag=f"wsb{wi}")
            for k0 in range(0, KB, step):
                k1 = min(KB, k0 + step)
                wdma = nc.scalar.dma_start(w_sb[:, k0:k1, :], wv[:, k0:k1, :])
                tile.add_dep_helper(
                    wdma.ins, ph0_dmas[-1].ins, reason="W loads after phase-0 x traffic"
                )
                chunk_no += 1
            w_sbs.append(w_sb)
        if bias_mode == "matmul":
            bias_row = wpool.tile([1, dout], f16)
            nc.scalar.dma_start(bias_row[:], bias[None, :])
            ones_t = wpool.tile([1, P], f16)
            nc.vector.memset(ones_t[:], 1.0)
        else:
            bias_bc = wpool.tile([P, dout], f32)
            nc.scalar.dma_start(bias_bc[0:1, :], bias[None, :])
            nc.gpsimd.partition_broadcast(bias_bc[:], bias_bc[0:1, :], channels=P)

        pk = spool.tile([P, 2], f32)
        nc.vector.tensor_reduce(pk[:, 0:1], smax[:], axis=AX.X, op=ALU.max)
        nc.vector.tensor_reduce(pk[:, 1:2], smin[:], axis=AX.X, op=ALU.min)
        nc.vector.tensor_scalar_mul(pk[:, 1:2], pk[:, 1:2], -1.0)
        pkr = spool.tile([P, 2], f32)
        nc.gpsimd.partition_all_reduce(
            pkr[:], pk[:], channels=P, reduce_op=bass_isa.ReduceOp.max
        )

        # ---- 8-core AllReduce(max) of [xmax, -xmin] ----
        sc = spool.tile([1, 2], f32)
        sem_in = nc.alloc_semaphore("ar_in")
        sem_cc = nc.alloc_semaphore("ar_cc")
        sem_out = nc.alloc_semaphore("ar_out")
        with tc.tile_critical():
            nc.gpsimd.dma_start(cc_in[None, :], pkr[0:1, :]).then_inc(sem_in, 16)
            nc.gpsimd.wait_ge(sem_in, 16)
            nc.gpsimd.collective_compute(
                "AllReduce",
                ALU.max,
                replica_groups=[list(range(n_cores))],
                ins=[cc_in],
                outs=[cc_out],
            ).then_inc(sem_cc, 1)
            nc.gpsimd.wait_ge(sem_cc, 1)
            nc.gpsimd.dma_start(sc[:], cc_out[None, :]).then_inc(sem_out, 16)
            nc.gpsimd.wait_ge(sem_out, 16)

        # ---- scalar math: inv_scale, zp ----
        scr = spool.tile([1, 6], f32)
        rng, inv, isc, nt, zp, mzp = (scr[0:1, i : i + 1] for i in range(6))
        nc.vector.tensor_add(rng, sc[0:1, 0:1], sc[0:1, 1:2])  # xmax - xmin
        nc.vector.reciprocal(inv, rng)
        nc.vector.tensor_scalar_mul(isc, inv, 255.0)  # 255/(xmax-xmin) ~ 1/scale
        nc.vector.tensor_mul(nt, sc[0:1, 1:2], isc)  # (-xmin)/scale
        # rne(nt); then zp = clip(-128 + rne(nt), -128, 127)
        nc.vector.tensor_scalar(zp, nt, MAGIC, -MAGIC, op0=ALU.add, op1=ALU.add)
        nc.vector.tensor_scalar(zp, zp, -128.0, -128.0, op0=ALU.add, op1=ALU.max)
        nc.vector.tensor_scalar_min(zp, zp, 127.0)
        nc.vector.tensor_scalar(mzp, zp, -1.0, MAGIC, op0=ALU.mult, op1=ALU.add)
        bc0 = spool.tile([1, 2], f32)
        nc.vector.tensor_copy(bc0[0:1, 0:1], isc)
        nc.vector.tensor_copy(bc0[0:1, 1:2], mzp)
        bc = spool.tile([P, 2], f32)
        nc.gpsimd.partition_broadcast(bc[:], bc0[:], channels=P)

        # ---- main loop: quantize + matmul per 128-token block ----
        for mb in range(MB):
            x_m = xpool.tile([P, KB * P], f32, tag="xm")
            x_m3 = x_m.rearrange("p (a b) -> p a b", b=P)  # [P, KB, P]
            nc.sync.dma_start(x_m3, x_view[:, :, mb * P : (mb + 1) * P])
            # v = x * inv_scale + MAGIC  (ACT); upper bits now hold rne(x/scale)
            nc.scalar.activation(x_m[:], x_m[:], AF.Copy, bias=MAGIC, scale=bc[:, 0:1])
            # v - (MAGIC - zp) = rne(x/scale) + zp ; clamp low
            nc.vector.tensor_scalar(
                x_m[:], x_m[:], bc[:, 1:2], QMIN, op0=ALU.subtract, op1=ALU.max
            )
            q_m = qpool.tile([P, KB, P], f16)
            nc.vector.tensor_scalar(q_m[:], x_m3, QMAX, None, op0=ALU.min)

            psum = ppool.tile([P, dout], f32)
            if bias_mode == "matmul":
                for n in range(NB):
                    nc.tensor.matmul(
                        psum[:, n * NMM : (n + 1) * NMM],
                        ones_t[:],
                        bias_row[:, n * NMM : (n + 1) * NMM],
                        start=True,
                        stop=False,
                    )
            last_wi = len(w_sbs) - 1
            for wi, w_sb in enumerate(w_sbs):
                for kb in range(KB):
                    lhsT = q_m[:, kb, :]
                    for n in range(NB):
                        nc.tensor.matmul(
                            psum[:, n * NMM : (n + 1) * NMM],
                            lhsT,
                            w_sb[:, kb, n * NMM : (n + 1) * NMM],
                            start=(bias_mode != "matmul" and wi == 0 and kb == 0),
                            stop=(kb == KB - 1 and wi == last_wi),
                        )
            o_m = opool.tile([P, dout], f32, tag="o_m")
            if bias_mode == "matmul":
                nc.scalar.copy(o_m[:], psum[:])
            else:
                nc.vector.scalar_tensor_tensor(
                    o_m[:], psum[:], 1.0, bias_bc[:], op0=ALU.mult, op1=ALU.add
                )
            nc.gpsimd.dma_start(y[mb * P : (mb + 1) * P, :], o_m[:])

    nc.compile()
    _dedupe_ldweights(nc)
    return nc


def _dedupe_ldweights(nc):
    """Remove back-to-back InstLdweights with identical weight access patterns.

    bacc's matmul split emits one Ldweights per Matmult even when consecutive
    matmuls share the stationary operand (our 4 n-slices per k-block). The PE
    keeps the stationary operand loaded between matmuls, so a repeat load with
    the same AP is pure overhead (~108ns each, ~half exposed). Only drop
    loads that carry no semaphore waits/updates.
    """
    from concourse import mybir

    for fn in nc.m.functions:
        for bb in fn.blocks:
            insts = bb.instructions
            keep = []
            last_ldw_key = None
            removed = 0
            for inst in insts:
                tname = type(inst).__name__
                if tname == "InstLdweights":
                    key = inst.concise()
                    if (
                        key == last_ldw_key
                        and not inst.has_wait()
                        and not inst.has_update()
                    ):
                        removed += 1
                        continue
                    last_ldw_key = key
                elif tname == "InstMatmult":
                    pass  # matmuls stream; they don't disturb loaded weights
                elif getattr(inst, "engine", None) == mybir.EngineType.PE and tname not in (
                    "InstEventSemaphore",
                    "InstNop",
                ):
                    # any other PE instruction: be conservative
                    last_ldw_key = None
                keep.append(inst)
            if removed:
                del insts[:]
                for inst in keep:
                    insts.append(inst)


def make_in_maps(
    x, weight, bias, r_shards=R_SHARDS, g_shards=G_SHARDS, w_passes=1, bias_mode="matmul"
):
    """Host-side shard/layout prep. Returns (in_maps, tok_c, dout_c)."""
    x = np.asarray(x, dtype=np.float32)
    weight = np.asarray(weight, dtype=np.float32)
    bias = np.asarray(bias, dtype=np.float32)
    tok_tot = int(np.prod(x.shape[:-1]))
    d_in = x.shape[-1]
    d_out = weight.shape[0]
    tok_c = tok_tot // r_shards
    dout_c = d_out // g_shards

    xt = np.ascontiguousarray(x.reshape(tok_tot, d_in).T)  # [d_in, tok_tot]
    b16 = bias.astype(np.float16) if bias_mode == "matmul" else bias.astype(np.float32)
    # g=1 cores get the d_in axis rolled by half so the SPMD program's
    # phase-0 min/max pass (which always scans the first d_in/2 rows) covers
    # the other half of x on the sibling core. Contraction order is
    # irrelevant to the matmul as long as xT and wT are rolled identically.
    half = d_in // 2

    def _roll(a, g):
        return a if g % 2 == 0 else np.concatenate([a[half:], a[:half]], axis=0)

    w_hi, w_lo = [], []
    for g in range(g_shards):
        wg = weight[g * dout_c : (g + 1) * dout_c, :].T  # [d_in, dout_c] fp32
        wg = _roll(wg, g)
        hi = wg.astype(np.float16)
        w_hi.append(np.ascontiguousarray(hi))
        if w_passes == 2:
            w_lo.append(np.ascontiguousarray((wg - hi.astype(np.float32)).astype(np.float16)))

    in_maps = []
    for c in range(r_shards * g_shards):
        r, g = divmod(c, g_shards)
        m = {
            "xT": np.ascontiguousarray(_roll(xt[:, r * tok_c : (r + 1) * tok_c], g)),
            "wT": w_hi[g],
            "bias": np.ascontiguousarray(b16[g * dout_c : (g + 1) * dout_c]),
        }
        if w_passes == 2:
            m["wLo"] = w_lo[g]
        in_maps.append(m)
    return in_maps, tok_c, dout_c


def assemble_output(results, out_shape, tok_c, dout_c, g_shards=G_SHARDS):
    d_out = out_shape[-1]
    tok_tot = int(np.prod(out_shape[:-1]))
    Y = np.empty((tok_tot, d_out), np.float32)
    for c, res in enumerate(results):
        r, g = divmod(c, g_shards)
        Y[r * tok_c : (r + 1) * tok_c, g * dout_c : (g + 1) * dout_c] = res["y"]
    return Y.reshape(out_shape)


_PROGRAM_CACHE = {}


def _get_program(d_in, tok_c, dout_c, w_passes, bias_mode):
    key = (d_in, tok_c, dout_c, w_passes, bias_mode)
    if key not in _PROGRAM_CACHE:
        _PROGRAM_CACHE[key] = build_program(
            d_in, tok_c, dout_c, N_CORES, w_passes, bias_mode
        )
    return _PROGRAM_CACHE[key]


def kernel(x, weight, bias, w_passes=1, bias_mode="matmul", trace=False):
    """Full-input entry point: shards across 8 NeuronCores, runs, gathers."""
    from concourse.bass_utils import run_bass_kernel_spmd

    assert x.shape == (B, S, D_IN) and weight.shape == (D_OUT, D_IN)
    in_maps, tok_c, dout_c = make_in_maps(
        x, weight, bias, w_passes=w_passes, bias_mode=bias_mode
    )
    nc = _get_program(D_IN, tok_c, dout_c, w_passes, bias_mode)
    out = run_bass_kernel_spmd(nc, in_maps, list(range(N_CORES)), trace=trace)
    res = assemble_output(out.results, (B, S, D_OUT), tok_c, dout_c)
    if trace:
        return res, out
    return res


# revision 28
# speedup vs baseline: 1.0635x; 1.0361x over previous
"""Trainium2 Bass kernel: per-tensor asymmetric int8 activation quantization
followed by a linear layer (y = quantize(x) @ W.T + bias).

Sharding (8 cores): 4-way over tokens x 2-way over out_features.
Each core receives:
  xT   [D_IN, TOK_C]   fp32  (x transposed, token-sharded)
  wT   [D_IN, DOUT_C]  fp16  (W transposed, out_feature-sharded)
  bias [DOUT_C]        fp16
and produces y [TOK_C, DOUT_C] fp32.

Device program per core:
  phase 0: streaming min/max over the local x shard (DVE reduce + GPSIMD
           partition all-reduce), then an 8-core AllReduce(max) of
           [xmax, -xmin] to get the global per-tensor range.
  scalars: inv_scale = 255/(xmax-xmin); zp = clip(-128 - rne(xmin/scale));
           rne() implemented with the fp32 magic constant 1.5*2^23.
  main:    for each 128-token block: quantize (ACT fused scale+magic, DVE
           zero-point + clip, cast fp16 -- q is integer in [-128,127] so
           fp16 is exact), then fp16 matmuls accumulate fp32 into PSUM with
           the weight tensor resident in SBUF; bias is folded in as a K=1
           matmul against a ones vector; result DMAd out in natural
           [token, dout] layout.
"""

import sys

import numpy as np

try:  # the grading environment may or may not have concourse on sys.path
    import concourse  # noqa: F401
except ImportError:  # pragma: no cover
    sys.path.insert(0, "/opt/trn_rl_repo")

P = 128
MAGIC = 12582912.0  # 1.5 * 2**23: fp32 add/sub rounds to nearest-even integer
QMIN, QMAX = -128.0, 127.0

# Full-problem shape (hardcoded per contract; kernel() checks them)
B, S, D_IN, D_OUT = 4, 2048, 4096, 4096
R_SHARDS, G_SHARDS = 4, 2  # token shards x out_feature shards
N_CORES = 8


def build_program(d_in, tok, dout, n_cores=N_CORES, w_passes=1, bias_mode="matmul"):
    """Emit the per-core SPMD program. Returns a compiled Bacc object.

    w_passes=2 adds a second accumulation pass against a residual weight
    input ("wLo") for near-fp32 weight precision at 2x PE cost.
    bias_mode: "matmul" folds bias in as a K=1 matmul; "evict" adds it
    during PSUM eviction on the vector engine (no K=1 weight loads).
    """
    from contextlib import ExitStack

    import concourse.bacc as bacc
    import concourse.tile as tile
    from concourse import bass_isa, mybir

    f32, f16 = mybir.dt.float32, mybir.dt.float16
    AF = mybir.ActivationFunctionType
    ALU = mybir.AluOpType
    AX = mybir.AxisListType

    assert d_in % P == 0 and tok % P == 0
    assert tok <= dout  # phase-0 reuses the [P, dout] output-pool slots
    KB, MB = d_in // P, tok // P
    KB0 = KB // 2  # phase-0 min/max half (the other half is on the g-sibling core)
    NMM = min(512, dout)
    assert dout % NMM == 0
    NB = dout // NMM

    nc = bacc.Bacc(
        "TRN2",
        target_bir_lowering=False,
        debug=False,
        num_devices=n_cores,
        enable_asserts=False,
    )

    xT = nc.dram_tensor("xT", [d_in, tok], f32, kind="ExternalInput").ap()
    wT = nc.dram_tensor("wT", [d_in, dout], f16, kind="ExternalInput").ap()
    bias = nc.dram_tensor("bias", [dout], f16, kind="ExternalInput").ap()
    w_ins = [wT]
    if w_passes == 2:
        w_ins.append(nc.dram_tensor("wLo", [d_in, dout], f16, kind="ExternalInput").ap())
    y = nc.dram_tensor("y", [tok, dout], f32, kind="ExternalOutput").ap()
    cc_in = nc.dram_tensor("cc_in", [2], f32).ap()
    cc_out = nc.dram_tensor("cc_out", [2], f32, addr_space="Shared").ap()

    x_view = xT.rearrange("(kb p) t -> p kb t", p=P)  # [P, KB, tok]
    w_views = [w.rearrange("(kb p) o -> p kb o", p=P) for w in w_ins]

    with tile.TileContext(nc) as tc, ExitStack() as ctx:
        wpool = ctx.enter_context(tc.tile_pool(name="w", bufs=1))
        xpool = ctx.enter_context(tc.tile_pool(name="x", bufs=2))
        qpool = ctx.enter_context(tc.tile_pool(name="q", bufs=2))
        opool = ctx.enter_context(tc.tile_pool(name="o", bufs=3))
        spool = ctx.enter_context(tc.tile_pool(name="s", bufs=1))
        ppool = ctx.enter_context(tc.tile_pool(name="ps", bufs=2, space="PSUM"))

        # ---- phase 0: min/max over the first half of this core's x shard
        # (the g-sibling core covers the other half; the host rolls the d_in
        # axis for g=1 cores so "first half" differs between siblings).
        # Full-row tiles: 8KB contiguous per partition -> full DMA rate.
        smax = spool.tile([P, KB0], f32)
        smin = spool.tile([P, KB0], f32)
        ph0_dmas = []
        for kb in range(KB0):
            # alternate pools for ~5 effective prefetch slots at no SBUF cost
            # (the q slots are [P, KB*P] fp16 = the same bytes as [P, tok] f32)
            if kb % 2 == 0:
                x_p = opool.tile([P, dout], f32, tag="o_m")
            else:
                x_p = qpool.tile([P, tok], f32, tag="q_m")
            x_row = x_p[:, 0:tok]
            ph0_dmas.append(nc.sync.dma_start(x_row, x_view[:, kb, :]))
            nc.vector.tensor_reduce(smax[:, kb : kb + 1], x_row, axis=AX.X, op=ALU.max)
            nc.vector.tensor_reduce(smin[:, kb : kb + 1], x_row, axis=AX.X, op=ALU.min)

        # Resident weights on the Scalar engine's HWDGE queue, staggered
        # behind phase-0's x traffic so they don't compete for HBM fabric.
        w_sbs = []
        n_chunks = 4
        step = max(1, KB // n_chunks)
        chunk_no = 0
        for wi, wv in enumerate(w_views):
            w_sb = wpool.tile([P, KB, dout], f16, tag=f"wsb{wi}")
            for k0 in range(0, KB, step):
                k1 = min(KB, k0 + step)
                wdma = nc.scalar.dma_start(w_sb[:, k0:k1, :], wv[:, k0:k1, :])
                tile.add_dep_helper(
                    wdma.ins, ph0_dmas[-1].ins, reason="W loads after phase-0 x traffic"
                )
                chunk_no += 1
            w_sbs.append(w_sb)
        if bias_mode == "matmul":
            bias_row = wpool.tile([1, dout], f16)
            nc.scalar.dma_start(bias_row[:], bias[None, :])
            ones_t = wpool.tile([1, P], f16)
            nc.vector.memset(ones_t[:], 1.0)
        else:
            bias_bc = wpool.tile([P, dout], f16)
            nc.scalar.dma_start(bias_bc[0:1, :], bias[None, :])
            nc.gpsimd.partition_broadcast(bias_bc[:], bias_bc[0:1, :], channels=P)

        pk = spool.tile([P, 2], f32)
        nc.vector.tensor_reduce(pk[:, 0:1], smax[:], axis=AX.X, op=ALU.max)
        nc.vector.tensor_reduce(pk[:, 1:2], smin[:], axis=AX.X, op=ALU.min)
        nc.vector.tensor_scalar_mul(pk[:, 1:2], pk[:, 1:2], -1.0)
        pkr = spool.tile([P, 2], f32)
        nc.gpsimd.partition_all_reduce(
            pkr[:], pk[:], channels=P, reduce_op=bass_isa.ReduceOp.max
        )

        # ---- 8-core AllReduce(max) of [xmax, -xmin] ----
        sc = spool.tile([1, 2], f32)
        sem_in = nc.alloc_semaphore("ar_in")
        sem_cc = nc.alloc_semaphore("ar_cc")
        sem_out = nc.alloc_semaphore("ar_out")
        with tc.tile_critical():
            nc.gpsimd.dma_start(cc_in[None, :], pkr[0:1, :]).then_inc(sem_in, 16)
            nc.gpsimd.wait_ge(sem_in, 16)
            nc.gpsimd.collective_compute(
                "AllReduce",
                ALU.max,
                replica_groups=[list(range(n_cores))],
                ins=[cc_in],
                outs=[cc_out],
            ).then_inc(sem_cc, 1)
            nc.gpsimd.wait_ge(sem_cc, 1)
            nc.gpsimd.dma_start(sc[:], cc_out[None, :]).then_inc(sem_out, 16)
            nc.gpsimd.wait_ge(sem_out, 16)

        # ---- scalar math: inv_scale, zp ----
        scr = spool.tile([1, 6], f32)
        rng, inv, isc, nt, zp, mzp = (scr[0:1, i : i + 1] for i in range(6))
        nc.vector.tensor_add(rng, sc[0:1, 0:1], sc[0:1, 1:2])  # xmax - xmin
        nc.vector.reciprocal(inv, rng)
        nc.vector.tensor_scalar_mul(isc, inv, 255.0)  # 255/(xmax-xmin) ~ 1/scale
        nc.vector.tensor_mul(nt, sc[0:1, 1:2], isc)  # (-xmin)/scale
        # rne(nt); then zp = clip(-128 + rne(nt), -128, 127)
        nc.vector.tensor_scalar(zp, nt, MAGIC, -MAGIC, op0=ALU.add, op1=ALU.add)
        nc.vector.tensor_scalar(zp, zp, -128.0, -128.0, op0=ALU.add, op1=ALU.max)
        nc.vector.tensor_scalar_min(zp, zp, 127.0)
        nc.vector.tensor_scalar(mzp, zp, -1.0, MAGIC, op0=ALU.mult, op1=ALU.add)
        bc0 = spool.tile([1, 2], f32)
        nc.vector.tensor_copy(bc0[0:1, 0:1], isc)
        nc.vector.tensor_copy(bc0[0:1, 1:2], mzp)
        bc = spool.tile([P, 2], f32)
        nc.gpsimd.partition_broadcast(bc[:], bc0[:], channels=P)

        # ---- main loop: quantize + matmul per 128-token block ----
        for mb in range(MB):
            x_m = xpool.tile([P, KB * P], f32, tag="xm")
            x_m3 = x_m.rearrange("p (a b) -> p a b", b=P)  # [P, KB, P]
            nc.sync.dma_start(x_m3, x_view[:, :, mb * P : (mb + 1) * P])
            # v = x * inv_scale + MAGIC  (ACT); upper bits now hold rne(x/scale)
            nc.scalar.activation(x_m[:], x_m[:], AF.Copy, bias=MAGIC, scale=bc[:, 0:1])
            # v - (MAGIC - zp) = rne(x/scale) + zp ; clamp low
            nc.vector.tensor_scalar(
                x_m[:], x_m[:], bc[:, 1:2], QMIN, op0=ALU.subtract, op1=ALU.max
            )
            q_m = qpool.tile([P, KB, P], f16)
            nc.vector.tensor_scalar(q_m[:], x_m3, QMAX, None, op0=ALU.min)

            psum = ppool.tile([P, dout], f32)
            if bias_mode == "matmul":
                for n in range(NB):
                    nc.tensor.matmul(
                        psum[:, n * NMM : (n + 1) * NMM],
                        ones_t[:],
                        bias_row[:, n * NMM : (n + 1) * NMM],
                        start=True,
                        stop=False,
                    )
            last_wi = len(w_sbs) - 1
            for wi, w_sb in enumerate(w_sbs):
                for kb in range(KB):
                    lhsT = q_m[:, kb, :]
                    for n in range(NB):
                        nc.tensor.matmul(
                            psum[:, n * NMM : (n + 1) * NMM],
                            lhsT,
                            w_sb[:, kb, n * NMM : (n + 1) * NMM],
                            start=(bias_mode != "matmul" and wi == 0 and kb == 0),
                            stop=(kb == KB - 1 and wi == last_wi),
                        )
            o_m = opool.tile([P, dout], f32, tag="o_m")
            if bias_mode == "matmul":
                nc.scalar.copy(o_m[:], psum[:])
            else:
                nc.vector.scalar_tensor_tensor(
                    o_m[:], psum[:], 1.0, bias_bc[:], op0=ALU.mult, op1=ALU.add
                )
            nc.gpsimd.dma_start(y[mb * P : (mb + 1) * P, :], o_m[:])

    nc.compile()
    _dedupe_ldweights(nc)
    return nc


def _dedupe_ldweights(nc):
    """Remove back-to-back InstLdweights with identical weight access patterns.

    bacc's matmul split emits one Ldweights per Matmult even when consecutive
    matmuls share the stationary operand (our 4 n-slices per k-block). The PE
    keeps the stationary operand loaded between matmuls, so a repeat load with
    the same AP is pure overhead (~108ns each, ~half exposed). Only drop
    loads that carry no semaphore waits/updates.
    """
    from concourse import mybir

    for fn in nc.m.functions:
        for bb in fn.blocks:
            insts = bb.instructions
            keep = []
            last_ldw_key = None
            removed = 0
            for inst in insts:
                tname = type(inst).__name__
                if tname == "InstLdweights":
                    key = inst.concise()
                    if (
                        key == last_ldw_key
                        and not inst.has_wait()
                        and not inst.has_update()
                    ):
                        removed += 1
                        continue
                    last_ldw_key = key
                elif tname == "InstMatmult":
                    pass  # matmuls stream; they don't disturb loaded weights
                elif getattr(inst, "engine", None) == mybir.EngineType.PE and tname not in (
                    "InstEventSemaphore",
                    "InstNop",
                ):
                    # any other PE instruction: be conservative
                    last_ldw_key = None
                keep.append(inst)
            if removed:
                del insts[:]
                for inst in keep:
                    insts.append(inst)


def make_in_maps(
    x, weight, bias, r_shards=R_SHARDS, g_shards=G_SHARDS, w_passes=1, bias_mode="matmul"
):
    """Host-side shard/layout prep. Returns (in_maps, tok_c, dout_c)."""
    x = np.asarray(x, dtype=np.float32)
    weight = np.asarray(weight, dtype=np.float32)
    bias = np.asarray(bias, dtype=np.float32)
    tok_tot = int(np.prod(x.shape[:-1]))
    d_in = x.shape[-1]
    d_out = weight.shape[0]
    tok_c = tok_tot // r_shards
    dout_c = d_out // g_shards

    xt = np.ascontiguousarray(x.reshape(tok_tot, d_in).T)  # [d_in, tok_tot]
    b16 = bias.astype(np.float16)
    # g=1 cores get the d_in axis rolled by half so the SPMD program's
    # phase-0 min/max pass (which always scans the first d_in/2 rows) covers
    # the other half of x on the sibling core. Contraction order is
    # irrelevant to the matmul as long as xT and wT are rolled identically.
    half = d_in // 2

    def _roll(a, g):
        return a if g % 2 == 0 else np.concatenate([a[half:], a[:half]], axis=0)

    w_hi, w_lo = [], []
    for g in range(g_shards):
        wg = weight[g * dout_c : (g + 1) * dout_c, :].T  # [d_in, dout_c] fp32
        wg = _roll(wg, g)
        hi = wg.astype(np.float16)
        w_hi.append(np.ascontiguousarray(hi))
        if w_passes == 2:
            w_lo.append(np.ascontiguousarray((wg - hi.astype(np.float32)).astype(np.float16)))

    in_maps = []
    for c in range(r_shards * g_shards):
        r, g = divmod(c, g_shards)
        m = {
            "xT": np.ascontiguousarray(_roll(xt[:, r * tok_c : (r + 1) * tok_c], g)),
            "wT": w_hi[g],
            "bias": np.ascontiguousarray(b16[g * dout_c : (g + 1) * dout_c]),
        }
        if w_passes == 2:
            m["wLo"] = w_lo[g]
        in_maps.append(m)
    return in_maps, tok_c, dout_c


def assemble_output(results, out_shape, tok_c, dout_c, g_shards=G_SHARDS):
    d_out = out_shape[-1]
    tok_tot = int(np.prod(out_shape[:-1]))
    Y = np.empty((tok_tot, d_out), np.float32)
    for c, res in enumerate(results):
        r, g = divmod(c, g_shards)
        Y[r * tok_c : (r + 1) * tok_c, g * dout_c : (g + 1) * dout_c] = res["y"]
    return Y.reshape(out_shape)


_PROGRAM_CACHE = {}


def _get_program(d_in, tok_c, dout_c, w_passes, bias_mode):
    key = (d_in, tok_c, dout_c, w_passes, bias_mode)
    if key not in _PROGRAM_CACHE:
        _PROGRAM_CACHE[key] = build_program(
            d_in, tok_c, dout_c, N_CORES, w_passes, bias_mode
        )
    return _PROGRAM_CACHE[key]


def kernel(x, weight, bias, w_passes=1, bias_mode="matmul", trace=False):
    """Full-input entry point: shards across 8 NeuronCores, runs, gathers."""
    from concourse.bass_utils import run_bass_kernel_spmd

    assert x.shape == (B, S, D_IN) and weight.shape == (D_OUT, D_IN)
    in_maps, tok_c, dout_c = make_in_maps(
        x, weight, bias, w_passes=w_passes, bias_mode=bias_mode
    )
    nc = _get_program(D_IN, tok_c, dout_c, w_passes, bias_mode)
    out = run_bass_kernel_spmd(nc, in_maps, list(range(N_CORES)), trace=trace)
    res = assemble_output(out.results, (B, S, D_OUT), tok_c, dout_c)
    if trace:
        return res, out
    return res
